# revision 5
# baseline (speedup 1.0000x reference)
"""Trainium2 Bass kernel for nn_MessagePassingLayer (GNN message passing).

Computes, for edges a[i] = (s, t) with edge features e[i] and node
features r:
    out = segment_sum(r[a[:,0]] * e, a[:,1]) + segment_sum(r[a[:,1]] * e, a[:,0])

Strategy (8 NeuronCores, full inputs in / full output out):
  - Expand each edge into its two messages (dst <- r[src] * e[edge]).
    Sort nodes by in-message count (degree) descending and give each node
    one SBUF partition row inside a block of 128 consecutive sorted nodes.
    Block j gets a fixed slot count K_j = max degree in the block; because
    nodes are degree-sorted, K_j is tight (a few % padding).
  - The host materializes r[src] and e[edge] into per-slot bf16 streams
    (rperm/eperm, laid out [128 partitions, sum_j K_j*D] per core), so the
    device only reads contiguous streams.
  - Device, per chunk of blocks: two DMA rings stream rperm/eperm, DVE
    multiplies them elementwise (bf16, 2x/4x mode), and the per-node sum
    over the K_j slots is an identity-matmul accumulation into PSUM
    (lhsT = I128, rhs = product slice k, fp32 accumulate) -- the Tensor
    engine does the segmented reduction with zero index machinery since
    slot k of node p lives in partition p.  PSUM is copied to bf16 via the
    Activation engine and written back with a third DMA ring.
  - Blocks are interleaved round-robin over cores (global block 8j+c ->
    core c local block j) so every core sees the same K_j schedule: one
    compiled program works for all 8 cores (SPMD), K_j = degree of the
    first node in global block 8j.
"""

import numpy as np

import concourse.bass as bass
import concourse.mybir as mybir
import concourse.tile as tile
from concourse.bass_utils import run_bass_kernel_spmd
from concourse.vector_clock import ScopedClock

P = 128
D = 128
N_CORES = 8
W_MAX = 8192   # max columns (bf16 elems) per streamed chunk per partition
MAXB = 6       # max blocks per chunk (each block holds a PSUM bank)

BF16 = mybir.dt.bfloat16
NP_BF16 = mybir.dt.np(BF16)

# ---------------------------------------------------------------------------
# Workarounds for the walrus build in this environment, which rejects any
# instruction carrying more than one semaphore wait ("Too many sync wait
# commands").  Tile's tail drain and scheduler can emit such instructions;
# split the extra waits onto dedicated single-wait NoOps.
# ---------------------------------------------------------------------------


def _patched_drain_and_barrier(self, tick_clock, wait_clock):
    nc = self.nc
    carrier = nc.sync.nop(nofuse=True, hint="drain_wait_carrier")
    wait_clock.add_sem_waits(carrier.ins, ScopedClock({None: tick_clock.global_clock}))
    si = carrier.ins.sync_info
    if si is not None and si.on_wait and len(si.on_wait) > 1:
        extras = list(si.on_wait[1:])
        del si.on_wait[1:]
        for w in extras:
            extra = nc.sync.nop(nofuse=True, hint="drain_wait_carrier")
            if extra.ins.sync_info is None:
                extra.ins.sync_info = mybir.SyncInfo(on_wait=[w], on_update=[])
            else:
                extra.ins.sync_info.on_wait.append(w)
    nc.sync.drain()
    nc.all_engine_barrier()
    assert self.sems is not None
    popped = nc._tile_sem_poison_stack.pop()
    assert popped is self._sem_poison
    nc.clear_and_free_semaphores(list(self.sems.allocated().values()))
    nc.all_engine_barrier()


tile.TileContext._drain_and_barrier = _patched_drain_and_barrier


def _split_multi_waits(nc):
    for fn in nc.m.functions:
        for bb in fn.blocks:
            out = []
            for inst in bb.instructions:
                si = inst.sync_info
                if si is not None and si.on_wait is not None and len(si.on_wait) > 1:
                    extras = list(si.on_wait[:-1])
                    del si.on_wait[:-1]
                    for w in extras:
                        out.append(mybir.InstNoOp(
                            text_hint="waitsplit",
                            bass_nofuse=True,
                            name=nc.get_next_instruction_name(),
                            engine=inst.engine,
                            ins=[], outs=[],
                            sync_info=mybir.SyncInfo(on_wait=[w], on_update=[]),
                        ))
                out.append(inst)
            bb.instructions[:] = out


# ---------------------------------------------------------------------------
# Device program
# ---------------------------------------------------------------------------


def build_kernel(meta, n_cores=N_CORES, iters=1):
    """meta["ks"]: per-local-block slot counts K_j (same for every core).
    Per-core inputs:
      rperm [P, S] bf16 : r[src] per slot; block j occupies columns
                          [off_j, off_j + K_j*D), slot k of node p at
                          rperm[p, off_j + k*D : off_j + (k+1)*D]
      eperm [P, S] bf16 : e[edge] per slot (0 at pad slots)
    Output: out [P, B*D] bf16 : columns [j*D,(j+1)*D) = features of block
    j's nodes (partition p = node p of the block).
    """
    ks = meta["ks"]
    B = len(ks)
    S = sum(k * D for k in ks)

    # chunk blocks: consecutive blocks, <= W_MAX columns, <= MAXB blocks
    chunks = []  # (col0, w, outcol0, [(K_j, local col off)...])
    j = 0
    col = 0
    outcol = 0
    while j < B:
        w = 0
        blks = []
        while j < B and len(blks) < MAXB and w + ks[j] * D <= W_MAX:
            blks.append((ks[j], w))
            w += ks[j] * D
            j += 1
        assert blks, f"block K={ks[j]} exceeds W_MAX"
        chunks.append((col, w, outcol, blks))
        col += w
        outcol += len(blks) * D

    nc = bass.Bass("TRN2", num_devices=n_cores)
    rperm_t = nc.declare_dram_parameter("rperm", [P, S], BF16, isOutput=False)
    eperm_t = nc.declare_dram_parameter("eperm", [P, S], BF16, isOutput=False)
    out_t = nc.declare_dram_parameter("out", [P, B * D], BF16, isOutput=True)

    with tile.TileContext(nc) as tc:
        with (
            tc.tile_pool(name="const", bufs=1) as constp,
            tc.tile_pool(name="rg", bufs=3) as rgp,
            tc.tile_pool(name="eg", bufs=3) as egp,
            tc.tile_pool(name="stage", bufs=3) as stagep,
            tc.tile_pool(name="psum", bufs=8, space="PSUM") as psump,
        ):
            # identity matrix for the per-node slot reduction
            col_i = constp.tile([P, P], mybir.dt.int32)
            nc.gpsimd.iota(col_i[:], pattern=[[1, P]], base=0,
                           channel_multiplier=0)
            part_i = constp.tile([P, P], mybir.dt.int32)
            nc.gpsimd.iota(part_i[:], pattern=[[0, P]], base=0,
                           channel_multiplier=1)
            ident = constp.tile([P, P], BF16)
            nc.vector.tensor_tensor(out=ident[:], in0=col_i[:], in1=part_i[:],
                                    op=mybir.AluOpType.is_equal)

            # spread the two load streams evenly over the three DMA queues
            qs = [nc.sync, nc.scalar, nc.gpsimd]
            for _ in range(iters):
                for ci, (col0, w, outcol0, blks) in enumerate(chunks):
                    rg = rgp.tile([P, W_MAX], BF16)
                    qs[(2 * ci) % 3].dma_start(rg[:, :w], rperm_t[:, col0:col0 + w])
                    eg = egp.tile([P, W_MAX], BF16)
                    qs[(2 * ci + 1) % 3].dma_start(eg[:, :w], eperm_t[:, col0:col0 + w])
                    nc.vector.tensor_mul(rg[:, :w], rg[:, :w], eg[:, :w])
                    stg = stagep.tile([P, MAXB * D], BF16)
                    for bi, (kj, boff) in enumerate(blks):
                        ps = psump.tile([P, P], mybir.dt.float32)
                        for k in range(kj):
                            c0 = boff + k * D
                            nc.tensor.matmul(
                                ps[:], lhsT=ident[:], rhs=rg[:, c0:c0 + D],
                                start=(k == 0), stop=(k == kj - 1))
                        nc.scalar.copy(stg[:, bi * D:(bi + 1) * D], ps[:])
                    nblk = len(blks)
                    nc.gpsimd.dma_start(
                        out_t[:, outcol0:outcol0 + nblk * D],
                        stg[:, :nblk * D])
    _split_multi_waits(nc)
    return nc


# ---------------------------------------------------------------------------
# Host-side sharding / layout
# ---------------------------------------------------------------------------


def preprocess(r, e, a, n_cores=N_CORES):
    """Returns (in_maps, meta). meta has the block schedule and the
    node-order info needed to assemble the full output."""
    r = np.ascontiguousarray(np.asarray(r), dtype=np.float32)
    e = np.ascontiguousarray(np.asarray(e), dtype=np.float32)
    a = np.asarray(a)
    N = r.shape[0]
    E = e.shape[0]
    s = a[:, 0].astype(np.int64)
    t = a[:, 1].astype(np.int64)
    dst = np.concatenate([t, s])
    src = np.concatenate([s, t])
    eid = np.concatenate([np.arange(E, dtype=np.int64)] * 2)
    M = dst.shape[0]

    deg = np.bincount(dst, minlength=N)
    order_nodes = np.argsort(-deg, kind="stable")   # degree descending
    pos_of_node = np.empty(N, dtype=np.int64)
    pos_of_node[order_nodes] = np.arange(N, dtype=np.int64)
    degs = deg[order_nodes]                          # per sorted position

    TBLK = -(-(-(-N // P)) // n_cores) * n_cores     # ceil(N/P) -> mult of 8
    Npad = TBLK * P
    B = TBLK // n_cores
    degs_pad = np.concatenate([degs, np.zeros(Npad - N, dtype=degs.dtype)])
    # K_j = max degree among global blocks 8j..8j+7 = first node's degree
    ks = [max(int(degs_pad[j * n_cores * P]), 1) for j in range(B)]
    S = sum(k * D for k in ks)
    off_slots = np.zeros(B + 1, dtype=np.int64)      # slot offset per block
    np.cumsum(np.asarray(ks, dtype=np.int64), out=off_slots[1:])

    # message -> (core, partition, slot index) in sorted-by-dst-position order
    morder = np.argsort(pos_of_node[dst], kind="stable")
    src_s = src[morder]
    eid_s = eid[morder]
    q = pos_of_node[dst][morder]                     # sorted position per msg
    start_q = np.concatenate([[0], np.cumsum(degs)]) # msg start per position
    within = np.arange(M, dtype=np.int64) - start_q[q]
    gblk = q // P
    p_row = q % P
    core = gblk % n_cores
    j_loc = gblk // n_cores
    slot = off_slots[j_loc] + within

    r16 = r.astype(NP_BF16)
    e16 = e.astype(NP_BF16)

    in_maps = []
    for c in range(n_cores):
        sel = core == c
        rp = np.zeros((P, off_slots[B], D), dtype=NP_BF16)
        ep = np.zeros((P, off_slots[B], D), dtype=NP_BF16)
        rp[p_row[sel], slot[sel]] = r16[src_s[sel]]
        ep[p_row[sel], slot[sel]] = e16[eid_s[sel]]
        in_maps.append({
            "rperm": rp.reshape(P, S),
            "eperm": ep.reshape(P, S),
        })

    meta = {
        "ks": ks,
        "B": B,
        "S": S,
        "N": N,
        "order_nodes": order_nodes,
        "n_cores": n_cores,
    }
    return in_maps, meta


def assemble(results, meta):
    N = meta["N"]
    B = meta["B"]
    n_cores = meta["n_cores"]
    order_nodes = meta["order_nodes"]
    out = np.empty((N, D), dtype=np.float32)
    qs = np.arange(N, dtype=np.int64)
    gblk = qs // P
    p_row = qs % P
    core = gblk % n_cores
    j_loc = gblk // n_cores
    for c in range(n_cores):
        sel = core == c
        oc = np.asarray(results[c]["out"]).reshape(P, B, D).astype(np.float32)
        out[order_nodes[qs[sel]]] = oc[p_row[sel], j_loc[sel]]
    return out


# ---------------------------------------------------------------------------
# Entry point
# ---------------------------------------------------------------------------


def kernel(r, e, a):
    in_maps, meta = preprocess(r, e, a, N_CORES)
    nc = build_kernel(meta, N_CORES, iters=1)
    res = run_bass_kernel_spmd(nc, in_maps, list(range(N_CORES)))
    return assemble(res.results, meta)


# revision 8
# speedup vs baseline: 1.3774x; 1.3774x over previous
"""Trainium2 Bass kernel for nn_MessagePassingLayer (GNN message passing).

Computes, for edges a[i] = (s, t) with edge features e[i] and node
features r:
    out = segment_sum(r[a[:,0]] * e, a[:,1]) + segment_sum(r[a[:,1]] * e, a[:,0])

Strategy (8 NeuronCores, full inputs in / full output out):
  - Expand each edge into its two messages (dst <- r[src] * e[edge]).
    Sort nodes by in-message count (degree) descending and give each node
    one SBUF partition row inside a block of 128 consecutive sorted nodes.
    Block j gets a fixed slot count K_j = max degree in the block; because
    nodes are degree-sorted, K_j is tight (a few % padding).
  - The host materializes r[src] and e[edge] into per-slot bf16 streams
    (rperm/eperm, laid out [128 partitions, sum_j K_j*D] per core), so the
    device only reads contiguous streams.
  - Device, per chunk of blocks: two DMA rings stream rperm/eperm, DVE
    multiplies them elementwise (bf16, 2x/4x mode), and the per-node sum
    over the K_j slots is an identity-matmul accumulation into PSUM
    (lhsT = I128, rhs = product slice k, fp32 accumulate) -- the Tensor
    engine does the segmented reduction with zero index machinery since
    slot k of node p lives in partition p.  PSUM is copied to bf16 via the
    Activation engine and written back with a third DMA ring.
  - Blocks are interleaved round-robin over cores (global block 8j+c ->
    core c local block j) so every core sees the same K_j schedule: one
    compiled program works for all 8 cores (SPMD), K_j = degree of the
    first node in global block 8j.
"""

import numpy as np

import concourse.bass as bass
import concourse.mybir as mybir
import concourse.tile as tile
from concourse.bass_utils import run_bass_kernel_spmd
from concourse.vector_clock import ScopedClock

P = 128
D = 128
N_CORES = 8
W_MAX = 8192   # max columns (bf16 elems) per streamed chunk per partition
MAXB = 6       # max blocks per chunk (each block holds a PSUM bank)

BF16 = mybir.dt.bfloat16
NP_BF16 = mybir.dt.np(BF16)

# ---------------------------------------------------------------------------
# Workarounds for the walrus build in this environment, which rejects any
# instruction carrying more than one semaphore wait ("Too many sync wait
# commands").  Tile's tail drain and scheduler can emit such instructions;
# split the extra waits onto dedicated single-wait NoOps.
# ---------------------------------------------------------------------------


def _patched_drain_and_barrier(self, tick_clock, wait_clock):
    nc = self.nc
    carrier = nc.sync.nop(nofuse=True, hint="drain_wait_carrier")
    wait_clock.add_sem_waits(carrier.ins, ScopedClock({None: tick_clock.global_clock}))
    si = carrier.ins.sync_info
    if si is not None and si.on_wait and len(si.on_wait) > 1:
        extras = list(si.on_wait[1:])
        del si.on_wait[1:]
        for w in extras:
            extra = nc.sync.nop(nofuse=True, hint="drain_wait_carrier")
            if extra.ins.sync_info is None:
                extra.ins.sync_info = mybir.SyncInfo(on_wait=[w], on_update=[])
            else:
                extra.ins.sync_info.on_wait.append(w)
    nc.sync.drain()
    nc.all_engine_barrier()
    assert self.sems is not None
    popped = nc._tile_sem_poison_stack.pop()
    assert popped is self._sem_poison
    nc.clear_and_free_semaphores(list(self.sems.allocated().values()))
    nc.all_engine_barrier()


tile.TileContext._drain_and_barrier = _patched_drain_and_barrier


def _split_multi_waits(nc):
    for fn in nc.m.functions:
        for bb in fn.blocks:
            out = []
            for inst in bb.instructions:
                si = inst.sync_info
                if si is not None and si.on_wait is not None and len(si.on_wait) > 1:
                    extras = list(si.on_wait[:-1])
                    del si.on_wait[:-1]
                    for w in extras:
                        out.append(mybir.InstNoOp(
                            text_hint="waitsplit",
                            bass_nofuse=True,
                            name=nc.get_next_instruction_name(),
                            engine=inst.engine,
                            ins=[], outs=[],
                            sync_info=mybir.SyncInfo(on_wait=[w], on_update=[]),
                        ))
                out.append(inst)
            bb.instructions[:] = out


# ---------------------------------------------------------------------------
# Device program
# ---------------------------------------------------------------------------


def build_kernel(meta, n_cores=N_CORES, iters=1):
    """meta["quads"]: list of (Qb, Kq, coloff, outcoloff) — Qb consecutive
    blocks sharing slot count Kq, laid out k-major:
      rperm[p, coloff + k*(Qb*D) + b*D + f] = slot k of node p of block b.
    Per-core inputs rperm/eperm [P, S] bf16 (eperm 0 at pad slots).
    Output: out [P, B*D] bf16, columns [j*D,(j+1)*D) = block j's features.
    One quad = one DMA pair + one DVE mult + Kq matmuls into one PSUM bank.
    """
    quads = meta["quads"]
    B = meta["B"]
    S = meta["S"]

    nc = bass.Bass("TRN2", num_devices=n_cores)
    rperm_t = nc.declare_dram_parameter("rperm", [P, S], BF16, isOutput=False)
    eperm_t = nc.declare_dram_parameter("eperm", [P, S], BF16, isOutput=False)
    out_t = nc.declare_dram_parameter("out", [P, B * D], BF16, isOutput=True)

    with tile.TileContext(nc) as tc:
        with (
            tc.tile_pool(name="const", bufs=1) as constp,
            tc.tile_pool(name="rg", bufs=3) as rgp,
            tc.tile_pool(name="eg", bufs=3) as egp,
            tc.tile_pool(name="stage", bufs=3) as stagep,
            tc.tile_pool(name="psum", bufs=6, space="PSUM") as psump,
        ):
            # identity matrix for the per-node slot reduction
            col_i = constp.tile([P, P], mybir.dt.int32)
            nc.gpsimd.iota(col_i[:], pattern=[[1, P]], base=0,
                           channel_multiplier=0)
            part_i = constp.tile([P, P], mybir.dt.int32)
            nc.gpsimd.iota(part_i[:], pattern=[[0, P]], base=0,
                           channel_multiplier=1)
            ident = constp.tile([P, P], BF16)
            nc.vector.tensor_tensor(out=ident[:], in0=col_i[:], in1=part_i[:],
                                    op=mybir.AluOpType.is_equal)

            for _ in range(iters):
                for (qb, kq, col0, outcol0) in quads:
                    w = kq * qb * D
                    rg = rgp.tile([P, W_MAX], BF16)
                    nc.sync.dma_start(rg[:, :w], rperm_t[:, col0:col0 + w])
                    eg = egp.tile([P, W_MAX], BF16)
                    nc.scalar.dma_start(eg[:, :w], eperm_t[:, col0:col0 + w])
                    nc.vector.tensor_mul(rg[:, :w], rg[:, :w], eg[:, :w])
                    qd = qb * D
                    ps = psump.tile([P, 4 * D], mybir.dt.float32)
                    for k in range(kq):
                        nc.tensor.matmul(
                            ps[:, :qd], lhsT=ident[:],
                            rhs=rg[:, k * qd:(k + 1) * qd],
                            start=(k == 0), stop=(k == kq - 1))
                    stg = stagep.tile([P, 4 * D], BF16)
                    nc.scalar.copy(stg[:, :qd], ps[:, :qd])
                    nc.gpsimd.dma_start(
                        out_t[:, outcol0:outcol0 + qd], stg[:, :qd])
    _split_multi_waits(nc)
    return nc


# ---------------------------------------------------------------------------
# Host-side sharding / layout
# ---------------------------------------------------------------------------


def preprocess(r, e, a, n_cores=N_CORES):
    """Returns (in_maps, meta). meta has the block schedule and the
    node-order info needed to assemble the full output."""
    r = np.ascontiguousarray(np.asarray(r), dtype=np.float32)
    e = np.ascontiguousarray(np.asarray(e), dtype=np.float32)
    a = np.asarray(a)
    N = r.shape[0]
    E = e.shape[0]
    s = a[:, 0].astype(np.int64)
    t = a[:, 1].astype(np.int64)
    dst = np.concatenate([t, s])
    src = np.concatenate([s, t])
    eid = np.concatenate([np.arange(E, dtype=np.int64)] * 2)
    M = dst.shape[0]

    deg = np.bincount(dst, minlength=N)
    order_nodes = np.argsort(-deg, kind="stable")   # degree descending
    pos_of_node = np.empty(N, dtype=np.int64)
    pos_of_node[order_nodes] = np.arange(N, dtype=np.int64)
    degs = deg[order_nodes]                          # per sorted position

    TBLK = -(-(-(-N // P)) // n_cores) * n_cores     # ceil(N/P) -> mult of 8
    Npad = TBLK * P
    B = TBLK // n_cores
    degs_pad = np.concatenate([degs, np.zeros(Npad - N, dtype=degs.dtype)])
    # K_j = max degree among global blocks 8j..8j+7 = first node's degree
    ks = [max(int(degs_pad[j * n_cores * P]), 1) for j in range(B)]

    # group consecutive local blocks into quads sharing Kq = K of the first
    # block; Q=2 while Kq*2*D > would exceed... use Q=4 when Kq<=16 (width
    # <= 8192 cols), else Q=2 (width <= 31*2*128 = 7936)
    quads = []            # (Qb, Kq, coloff_cols, outcoloff_cols)
    blk_quad = np.zeros(B, dtype=np.int64)   # block -> quad index
    b_in_quad = np.zeros(B, dtype=np.int64)
    j = 0
    col = 0
    while j < B:
        kq = ks[j]
        qmax = 4 if kq <= 16 else 2
        qb = min(qmax, B - j)
        for bb in range(qb):
            blk_quad[j + bb] = len(quads)
            b_in_quad[j + bb] = bb
        quads.append((qb, kq, col, j * D))
        col += kq * qb * D
        j += qb
    S = col
    quad_coloff = np.asarray([q[2] for q in quads], dtype=np.int64)
    quad_qb = np.asarray([q[0] for q in quads], dtype=np.int64)

    # message -> (core, partition, column) in sorted-by-dst-position order
    morder = np.argsort(pos_of_node[dst], kind="stable")
    src_s = src[morder]
    eid_s = eid[morder]
    q = pos_of_node[dst][morder]                     # sorted position per msg
    start_q = np.concatenate([[0], np.cumsum(degs)]) # msg start per position
    within = np.arange(M, dtype=np.int64) - start_q[q]
    gblk = q // P
    p_row = q % P
    core = gblk % n_cores
    j_loc = gblk // n_cores
    qd_of = blk_quad[j_loc]
    # slot index in D-column units: quad offset + k*(Qb) + b_in_quad
    slot = (quad_coloff[qd_of] // D + within * quad_qb[qd_of]
            + b_in_quad[j_loc])

    r16 = r.astype(NP_BF16)
    e16 = e.astype(NP_BF16)

    nslots = S // D
    in_maps = []
    for c in range(n_cores):
        sel = core == c
        rp = np.zeros((P, nslots, D), dtype=NP_BF16)
        ep = np.zeros((P, nslots, D), dtype=NP_BF16)
        rp[p_row[sel], slot[sel]] = r16[src_s[sel]]
        ep[p_row[sel], slot[sel]] = e16[eid_s[sel]]
        in_maps.append({
            "rperm": rp.reshape(P, S),
            "eperm": ep.reshape(P, S),
        })

    meta = {
        "ks": ks,
        "quads": quads,
        "B": B,
        "S": S,
        "N": N,
        "order_nodes": order_nodes,
        "n_cores": n_cores,
    }
    return in_maps, meta


def assemble(results, meta):
    N = meta["N"]
    B = meta["B"]
    n_cores = meta["n_cores"]
    order_nodes = meta["order_nodes"]
    out = np.empty((N, D), dtype=np.float32)
    qs = np.arange(N, dtype=np.int64)
    gblk = qs // P
    p_row = qs % P
    core = gblk % n_cores
    j_loc = gblk // n_cores
    for c in range(n_cores):
        sel = core == c
        oc = np.asarray(results[c]["out"]).reshape(P, B, D).astype(np.float32)
        out[order_nodes[qs[sel]]] = oc[p_row[sel], j_loc[sel]]
    return out


# ---------------------------------------------------------------------------
# Entry point
# ---------------------------------------------------------------------------


def kernel(r, e, a):
    in_maps, meta = preprocess(r, e, a, N_CORES)
    nc = build_kernel(meta, N_CORES, iters=1)
    res = run_bass_kernel_spmd(nc, in_maps, list(range(N_CORES)))
    return assemble(res.results, meta)
